# revision 1
# baseline (speedup 1.0000x reference)
"""Trainium2 Bass kernel for nn_DirectRecurrentODE (spline-driven RK4 ODE).

Computation (mirrors the reference):
  X(t): natural cubic spline over per-batch coeffs; f(t,z) = 2-layer tanh MLP
  on [z, X(t)]; rk4 3/8-rule scan over times=arange(512); per-batch
  final_index gather; linear readout.

Mapping (latency-optimized: total time ~= 511 x per-step serial latency):
- Data-parallel over batch: 512 -> 8 cores x 64; one 64-wide chain per core;
  channels on partitions, batch on free dim.
- Per-step critical path is exactly 4 evals x [tanh_k -> W1z-variant matmul
  -> tanh_h -> W2 matmul] (8 matmuls + 8 tanh + 16 semaphore hops). HW
  matmuls cost ~3x the cost model (~300ns marginal, measured), so everything
  else is kept OFF both the path and the PE:
  * spline E-terms (host-precomputed W1x^T X, streamed) enter via DVE: the
    eval-1 term seeds the fresh PSUM bank with a plain DVE copy and the
    spine matmuls accumulate on top with start=False (keeps the seed off the
    critical path); later eval deltas are DVE adds after each tanh_h read.
  * RK4 k-combinations: the one new k-term per eval is a pre-scaled W1z
    matmul on the path; cross-eval terms (k1, k2 reuses) are extra pre-scaled
    matmuls issued right after their k is ready (PE slack).
  * z' update (3/8-rule) and zp2 = z' - k4/8 on DVE (off-path); the next
    step's spine is W1z @ zp2 (early) + (W1z/8) @ k4 (path).
  * final_index gather: DVE mask-select + accumulate into zT each step.
- Host: float64 spline tables, E einsum, weight variant pre-scaling,
  shard/unshard.
"""
import sys
import numpy as np

for _p in ("/opt/trn_rl_repo",):
    if _p not in sys.path:
        sys.path.append(_p)

import concourse.bass as bass
import concourse.bacc as bacc
import concourse.tile as tile
from concourse import mybir
from concourse.bass_utils import run_bass_kernel_spmd
from concourse import dve_ops
from concourse.dve_spec import Spec, Src0, Src1, C0, Zero, eq, select, lower
from concourse.dve_uop import DveOpSpec

F32 = mybir.dt.float32
AFT = mybir.ActivationFunctionType

B, L, C_IN, C_HID, C_HH, C_OUT = 512, 512, 32, 64, 128, 10
N_CORES = 8
BC = B // N_CORES
T_FULL = L - 1
CHUNK = 16


def _register_dve_op(name, spec, subdim=False):
    for op in dve_ops.OPS:
        if op.name == name:
            return op
    opcode = max(dve_ops._SUB_OPCODE_FOR_NAME.values()) + 1
    assert opcode < 0x20
    shas = {}
    for ver in ("v3", "v4"):
        try:
            uops = lower(spec, ver=ver)
            shas[ver] = DveOpSpec(
                name=name, opcode=opcode, uops=uops,
                rd1_en=dve_ops.has_src1(spec),
            ).sha(ver)
        except Exception:
            pass
    op = dve_ops.DveOp(name, spec, subdim=subdim, uops_sha=shas)
    dve_ops.OPS.append(op)
    dve_ops._SUB_OPCODE_FOR_NAME[name] = opcode
    dve_ops.CUSTOM_DVE_SPECS[name] = spec
    return op


AXPY = _register_dve_op(
    "ANT_AXPY",
    Spec(body=Src0 + C0 * Src1,
         reference=lambda in0, in1, c0, c1, c2: in0 + c0 * in1),
)

MASKSEL = _register_dve_op(
    "ANT_MASKSEL",
    Spec(body=select(eq(Src1, C0), Src0, Zero),
         reference=lambda in0, in1, c0, c1, c2: np.where(in1 == c0, in0, 0.0)),
)


def _spline_tables(times, a, b, c, d):
    a = np.asarray(a, np.float64)
    b_ = np.asarray(b, np.float64)
    c_ = np.asarray(c, np.float64)
    d_ = np.asarray(d, np.float64)
    tail = (a[:, -1] + b_[:, -1] + 0.5 * c_[:, -1] + d_[:, -1] / 3.0)[:, None]
    A = np.concatenate([a, tail], axis=1)
    X13 = a + b_ / 3.0 + c_ / 18.0 + d_ / 81.0
    X23 = a + (2.0 / 3.0) * b_ + (2.0 / 9.0) * c_ + (8.0 / 81.0) * d_
    return A, X13, X23


def build_program(T=T_FULL, b1_nonzero=False, repeats=1):
    nc = bacc.Bacc()
    n_chunks = (T + CHUNK - 1) // CHUNK
    t_pad = n_chunks * CHUNK

    cf_in = nc.declare_dram_parameter("cf", [C_HH, t_pad, 4, BC], F32, isOutput=False)
    a0_in = nc.declare_dram_parameter("a0", [C_IN, BC], F32, isOutput=False)
    # slabs [64, C_HH]: W1z, W1z/8, W1z/3, -2/3 W1z, 4/3 W1z, -2 W1z
    wz_in = nc.declare_dram_parameter("wz", [6, C_HID, C_HH], F32, isOutput=False)
    w1x_in = nc.declare_dram_parameter("w1x", [C_IN, C_HH], F32, isOutput=False)
    w2_in = nc.declare_dram_parameter("w2", [C_HH, C_HID], F32, isOutput=False)
    winit_in = nc.declare_dram_parameter("winit", [C_IN, C_HID], F32, isOutput=False)
    wout_in = nc.declare_dram_parameter("wout", [C_HID, C_OUT], F32, isOutput=False)
    bvec_in = nc.declare_dram_parameter("bvec", [4, 128], F32, isOutput=False)
    fi_in = nc.declare_dram_parameter("fi", [C_HID, BC], F32, isOutput=False)
    out_ext = nc.declare_dram_parameter("out", [C_OUT, BC], F32, isOutput=True)

    import contextlib
    with tile.TileContext(nc) as tc, contextlib.ExitStack() as ctx:
        singles = ctx.enter_context(tc.tile_pool(name="singles", bufs=1))
        cf_pool = ctx.enter_context(tc.tile_pool(name="cf", bufs=3))
        hpool = ctx.enter_context(tc.tile_pool(name="hpool", bufs=3))
        kpool = ctx.enter_context(tc.tile_pool(name="kpool", bufs=2))
        zpool = ctx.enter_context(tc.tile_pool(name="zpool", bufs=2))
        gpool = ctx.enter_context(tc.tile_pool(name="gpool", bufs=2))
        p1pool = ctx.enter_context(tc.tile_pool(name="p1", bufs=3, space="PSUM"))
        p2pool = ctx.enter_context(tc.tile_pool(name="p2", bufs=4, space="PSUM"))

        a0t = singles.tile([128, BC], F32)
        nc.sync.dma_start(out=a0t[0:C_IN, :], in_=a0_in[:, :])
        wz = []
        for v in range(6):
            wv = singles.tile([128, C_HH], F32, name=f"wz{v}")
            nc.sync.dma_start(out=wv[0:C_HID, :], in_=wz_in[v, :, :])
            wz.append(wv)
        W_1, W_18, W_13, W_M23, W_43, W_M2 = wz
        w1x = singles.tile([128, C_HH], F32)
        nc.sync.dma_start(out=w1x[0:C_IN, :], in_=w1x_in[:, :])
        w2 = singles.tile([128, C_HID], F32)
        nc.sync.dma_start(out=w2[:, :], in_=w2_in[:, :])
        winit = singles.tile([128, C_HID], F32)
        nc.sync.dma_start(out=winit[0:C_IN, :], in_=winit_in[:, :])
        wout = singles.tile([128, C_OUT], F32)
        nc.sync.dma_start(out=wout[0:64, :], in_=wout_in[:, :])
        bvec = singles.tile([128, 4], F32)
        for r in range(4):
            nc.sync.dma_start(out=bvec[:, r:r + 1],
                              in_=bvec_in[r:r + 1, :].rearrange("o p -> p o"))
        fi_rep = singles.tile([128, BC], F32)
        nc.sync.dma_start(out=fi_rep[0:64, :], in_=fi_in[:, :])

        zT = singles.tile([128, BC], F32)
        nc.vector.memset(zT[0:64, :], 0.0)

        def load_chunk(chk):
            cft = cf_pool.tile([128, CHUNK * 4 * BC], F32, name="cft", tag="cft")
            nc.sync.dma_start(
                out=cft[:, :].rearrange("c (t e b) -> c t e b", t=CHUNK, e=4),
                in_=cf_in[:, chk * CHUNK:(chk + 1) * CHUNK, :, :],
            )
            return cft

        cft = load_chunk(0)

        # ---- z0 ----
        p0 = p1pool.tile([128, BC], F32, name="p1t", tag="p1")
        nc.tensor.matmul(p0[0:64, :], winit[0:C_IN, :], a0t[0:C_IN, :],
                         start=True, stop=True, tile_position=(0, 0))
        z = zpool.tile([128, BC], F32, name="z", tag="z")
        nc.scalar.activation(z[0:64, :], p0[0:64, :], AFT.Identity,
                             bias=bvec[0:64, 2:3])
        g0 = gpool.tile([128, BC], F32, name="g", tag="g")
        nc.vector._custom_dve(MASKSEL, out=g0[0:64, :], in0=z[0:64, :],
                              in1=fi_rep[0:64, :], s0=0.0)
        nc.vector.tensor_add(zT[0:64, :], zT[0:64, :], g0[0:64, :])

        k4_prev = None

        b1bias = bvec[0:128, 0:1]

        # "front half" of a step: open its p1 bank with the early zp2 spine
        # matmul and RMW-add the E0 term on DVE. Pre-emitted during eval 4 of
        # the PREVIOUS step (right after zp2 is computed, before any DVE op
        # that waits on tanh_k4) so both stay off the critical path and the
        # only path op at the step boundary is the W1z/8 @ k4 matmul.
        _loaded_first = [False]

        def front_half(step_idx, zsrc):
            nonlocal cft
            if step_idx % CHUNK == 0 and _loaded_first[0]:
                cft = load_chunk(step_idx // CHUNK)
            _loaded_first[0] = True
            base = (step_idx % CHUNK) * 4 * BC
            p1n = p1pool.tile([128, BC], F32, name="p1t", tag="p1")
            nc.tensor.matmul(p1n[:, :], W_1[0:64, :], zsrc[0:64, :],
                             start=True, stop=False, tile_position=(0, 0))
            nc.vector.tensor_add(p1n[:, :], p1n[:, :],
                                 cft[:, base:base + BC])
            return p1n, cft, base

        p1_state = front_half(0, z)   # step 0: zp2_prev := z0, no k4 term

        for _rep in range(repeats):
            for t in range(T):
                p1, bcft, base = p1_state

                def xs(e, _c=bcft, _b=base):
                    return _c[:, _b + e * BC: _b + (e + 1) * BC]

                if k4_prev is not None:
                    nc.tensor.matmul(p1[:, :], W_18[0:64, :], k4_prev[0:64, :],
                                     start=False, stop=False, tile_position=(0, 0))

                k = [None] * 4
                h = [None] * 4
                q = [None] * 4

                # ---- eval 1 ----
                h[0] = hpool.tile([128, BC], F32, name="h", tag="h")
                nc.scalar.activation(h[0][:, :], p1[:, :], AFT.Tanh, bias=b1bias)
                q[0] = p2pool.tile([128, BC], F32, name="p2t", tag="p2")
                nc.tensor.matmul(q[0][0:64, :], w2[:, :], h[0][:, :],
                                 start=True, stop=True, tile_position=(0, 0))
                nc.vector.tensor_add(p1[:, :], p1[:, :], xs(1))
                k[0] = kpool.tile([128, BC], F32, name="k1", tag="k1")
                nc.scalar.activation(k[0][0:64, :], q[0][0:64, :], AFT.Tanh,
                                     bias=bvec[0:64, 1:2])
                # ---- eval 2 ----
                nc.tensor.matmul(p1[:, :], W_13[0:64, :], k[0][0:64, :],
                                 start=False, stop=False, tile_position=(0, 0))
                h[1] = hpool.tile([128, BC], F32, name="h", tag="h")
                nc.scalar.activation(h[1][:, :], p1[:, :], AFT.Tanh, bias=b1bias)
                q[1] = p2pool.tile([128, BC], F32, name="p2t", tag="p2")
                nc.tensor.matmul(q[1][0:64, :], w2[:, :], h[1][:, :],
                                 start=True, stop=True, tile_position=(0, 0))
                nc.vector.tensor_add(p1[:, :], p1[:, :], xs(2))
                nc.tensor.matmul(p1[:, :], W_M23[0:64, :], k[0][0:64, :],
                                 start=False, stop=False, tile_position=(0, 0))
                k[1] = kpool.tile([128, BC], F32, name="k2", tag="k2")
                nc.scalar.activation(k[1][0:64, :], q[1][0:64, :], AFT.Tanh,
                                     bias=bvec[0:64, 1:2])
                # ---- eval 3 ----
                nc.tensor.matmul(p1[:, :], W_1[0:64, :], k[1][0:64, :],
                                 start=False, stop=False, tile_position=(0, 0))
                h[2] = hpool.tile([128, BC], F32, name="h", tag="h")
                nc.scalar.activation(h[2][:, :], p1[:, :], AFT.Tanh, bias=b1bias)
                q[2] = p2pool.tile([128, BC], F32, name="p2t", tag="p2")
                nc.tensor.matmul(q[2][0:64, :], w2[:, :], h[2][:, :],
                                 start=True, stop=True, tile_position=(0, 0))
                nc.vector.tensor_add(p1[:, :], p1[:, :], xs(3))
                nc.tensor.matmul(p1[:, :], W_43[0:64, :], k[0][0:64, :],
                                 start=False, stop=False, tile_position=(0, 0))
                nc.tensor.matmul(p1[:, :], W_M2[0:64, :], k[1][0:64, :],
                                 start=False, stop=False, tile_position=(0, 0))
                k[2] = kpool.tile([128, BC], F32, name="k3", tag="k3")
                nc.scalar.activation(k[2][0:64, :], q[2][0:64, :], AFT.Tanh,
                                     bias=bvec[0:64, 1:2])
                # ---- eval 4 ----
                nc.tensor.matmul(p1[:, :], W_1[0:64, :], k[2][0:64, :],
                                 start=False, stop=True, tile_position=(0, 0))
                # z-update front half on DVE (off-path): zp2 = z + (k1+3k2+3k3)/8
                s2 = hpool.tile([128, BC], F32, name="s2", tag="s2")
                nc.vector.tensor_add(s2[0:64, :], k[1][0:64, :], k[2][0:64, :])
                zp = hpool.tile([128, BC], F32, name="zp", tag="zp")
                nc.vector._custom_dve(AXPY, out=zp[0:64, :], in0=z[0:64, :],
                                      in1=s2[0:64, :], s0=0.375)
                zp2 = zpool.tile([128, BC], F32, name="zp2", tag="zp2")
                nc.vector._custom_dve(AXPY, out=zp2[0:64, :], in0=zp[0:64, :],
                                      in1=k[0][0:64, :], s0=0.125)
                if not (_rep == repeats - 1 and t == T - 1):
                    p1_state = front_half(t + 1 if t + 1 < T else 0, zp2)
                h[3] = hpool.tile([128, BC], F32, name="h", tag="h")
                nc.scalar.activation(h[3][:, :], p1[:, :], AFT.Tanh, bias=b1bias)
                q[3] = p2pool.tile([128, BC], F32, name="p2t", tag="p2")
                nc.tensor.matmul(q[3][0:64, :], w2[:, :], h[3][:, :],
                                 start=True, stop=True, tile_position=(0, 0))
                k[3] = kpool.tile([128, BC], F32, name="k4", tag="k4")
                nc.scalar.activation(k[3][0:64, :], q[3][0:64, :], AFT.Tanh,
                                     bias=bvec[0:64, 1:2])
                # z' = zp2 + k4/8 ; gather
                znew = zpool.tile([128, BC], F32, name="z", tag="z")
                nc.vector._custom_dve(AXPY, out=znew[0:64, :], in0=zp2[0:64, :],
                                      in1=k[3][0:64, :], s0=0.125)
                g = gpool.tile([128, BC], F32, name="g", tag="g")
                nc.vector._custom_dve(MASKSEL, out=g[0:64, :], in0=znew[0:64, :],
                                      in1=fi_rep[0:64, :], s0=float(t + 1))
                nc.vector.tensor_add(zT[0:64, :], zT[0:64, :], g[0:64, :])

                z = znew
                k4_prev = k[3]

        # ---- readout ----
        po = p2pool.tile([128, BC], F32, name="po", tag="p2")
        nc.tensor.matmul(po[0:C_OUT, :], wout[0:64, :], zT[0:64, :],
                         start=True, stop=True, tile_position=(0, 0))
        ot = singles.tile([128, BC], F32)
        nc.scalar.activation(ot[0:C_OUT, :], po[0:C_OUT, :], AFT.Identity,
                             bias=bvec[0:C_OUT, 3:4])
        nc.sync.dma_start(out=out_ext[:, :], in_=ot[0:C_OUT, :])

    nc.compile()
    return nc


def prepare_inputs(times, coeff_a, coeff_b, coeff_two_c, coeff_three_d,
                   final_index, W_init, b_init, W1, b1, W2, b2, W_out, b_out,
                   T=T_FULL):
    fi = np.asarray(final_index).astype(np.int64)
    W1 = np.asarray(W1, np.float64)
    b1 = np.asarray(b1, np.float32)
    W2_ = np.asarray(W2, np.float32)
    b2_ = np.asarray(b2, np.float32)
    W_init_ = np.asarray(W_init, np.float32)
    b_init_ = np.asarray(b_init, np.float32)
    W_out_ = np.asarray(W_out, np.float32)
    b_out_ = np.asarray(b_out, np.float32)

    A, X13, X23 = _spline_tables(times, coeff_a, coeff_b, coeff_two_c,
                                 coeff_three_d)
    b1_nonzero = bool(np.any(b1 != 0))
    n_chunks = (T + CHUNK - 1) // CHUNK
    t_pad = n_chunks * CHUNK

    At = np.transpose(A, (2, 1, 0))
    X13t = np.transpose(X13, (2, 1, 0))
    X23t = np.transpose(X23, (2, 1, 0))
    Xd = np.zeros((C_IN, t_pad, 4, B), np.float64)
    Xd[:, :T, 0] = At[:, :T]
    Xd[:, :T, 1] = (X13t - At[:, :L - 1])[:, :T]
    Xd[:, :T, 2] = (X23t - X13t)[:, :T]
    Xd[:, :T, 3] = (At[:, 1:] - X23t)[:, :T]
    W1x64 = W1[C_HID:]
    E = np.einsum("cteb,ch->hteb", Xd, W1x64, optimize=True)
    cf_all = np.ascontiguousarray(E, np.float32)
    a0_all = np.ascontiguousarray(At[:, 0], np.float32)

    W1z = W1[:C_HID]
    W1x_ = np.ascontiguousarray(W1[C_HID:].astype(np.float32))

    wz = np.stack([W1z, W1z / 8.0, W1z / 3.0, -2.0 / 3.0 * W1z,
                   4.0 / 3.0 * W1z, -2.0 * W1z]).astype(np.float32)

    bvec = np.zeros((4, 128), np.float32)
    bvec[0, :C_HH] = b1
    bvec[1, :C_HID] = b2_
    bvec[2, :C_HID] = b_init_
    bvec[3, :C_OUT] = b_out_

    in_maps = []
    for core in range(N_CORES):
        cols = slice(core * BC, (core + 1) * BC)
        in_maps.append({
            "cf": np.ascontiguousarray(cf_all[:, :, :, cols]),
            "a0": np.ascontiguousarray(a0_all[:, cols]),
            "wz": wz,
            "w1x": W1x_,
            "w2": W2_,
            "winit": W_init_,
            "wout": W_out_,
            "bvec": bvec,
            "fi": np.ascontiguousarray(
                np.broadcast_to(fi[cols].astype(np.float32), (C_HID, BC))),
        })
    return in_maps, b1_nonzero


_PROGRAM_CACHE = {}


def run(inputs, T=T_FULL, trace=False):
    in_maps, b1_nonzero = prepare_inputs(T=T, **inputs)
    key = (T, b1_nonzero)
    if key not in _PROGRAM_CACHE:
        _PROGRAM_CACHE[key] = build_program(T=T, b1_nonzero=b1_nonzero)
    nc = _PROGRAM_CACHE[key]
    res = run_bass_kernel_spmd(nc, in_maps, core_ids=list(range(N_CORES)),
                               trace=trace)
    outs = [res.results[c]["out"] for c in range(N_CORES)]
    full = np.concatenate([o.T for o in outs], axis=0).astype(np.float32)
    return full, res


def kernel(**inputs):
    out, _ = run(inputs)
    return out



# revision 2
# speedup vs baseline: 1.2929x; 1.2929x over previous
"""Trainium2 Bass kernel for nn_DirectRecurrentODE (v3).

Changes vs v1: zp2 = z + (k1+3k2+3k3)/8 is built incrementally (one AXPY
after each k tanh) so it is ready one DVE op after tanh-k3, and front_half
is issued after the path W2.h4 matmul so the in-order PE queue cannot stall
the path behind the zp2-dependent spine opener. E-adds stay on DVE (the
E-on-PE variant measured much slower: extra per-matmul weight loads
overflow the PE idle windows between path matmuls).

Computation (mirrors the reference):
  X(t): natural cubic spline over per-batch coeffs; f(t,z) = 2-layer tanh MLP
  on [z, X(t)]; rk4 3/8-rule scan over times=arange(512); per-batch
  final_index gather; linear readout.

Mapping (latency-optimized: total time ~= 511 x per-step serial latency):
- Data-parallel over batch: 512 -> 8 cores x 64; one 64-wide chain per core;
  channels on partitions, batch on free dim.
- Per-step critical path is exactly 4 evals x [tanh_k -> W1z-variant matmul
  -> tanh_h -> W2 matmul] (8 matmuls + 8 tanh + 16 semaphore hops). HW
  matmuls cost ~3x the cost model (~300ns marginal, measured), so everything
  else is kept OFF both the path and the PE:
  * spline E-terms (host-precomputed W1x^T X, streamed) enter via DVE: the
    eval-1 term seeds the fresh PSUM bank with a plain DVE copy and the
    spine matmuls accumulate on top with start=False (keeps the seed off the
    critical path); later eval deltas are DVE adds after each tanh_h read.
  * RK4 k-combinations: the one new k-term per eval is a pre-scaled W1z
    matmul on the path; cross-eval terms (k1, k2 reuses) are extra pre-scaled
    matmuls issued right after their k is ready (PE slack).
  * z' update (3/8-rule) and zp2 = z' - k4/8 on DVE (off-path); the next
    step's spine is W1z @ zp2 (early) + (W1z/8) @ k4 (path).
  * final_index gather: DVE mask-select + accumulate into zT each step.
- Host: float64 spline tables, E einsum, weight variant pre-scaling,
  shard/unshard.
"""
import sys
import numpy as np

for _p in ("/opt/trn_rl_repo",):
    if _p not in sys.path:
        sys.path.append(_p)

import concourse.bass as bass
import concourse.bacc as bacc
import concourse.tile as tile
from concourse import mybir
from concourse.bass_utils import run_bass_kernel_spmd
from concourse import dve_ops
from concourse.dve_spec import Spec, Src0, Src1, C0, Zero, eq, select, lower
from concourse.dve_uop import DveOpSpec

F32 = mybir.dt.float32
AFT = mybir.ActivationFunctionType

B, L, C_IN, C_HID, C_HH, C_OUT = 512, 512, 32, 64, 128, 10
N_CORES = 8
BC = B // N_CORES
T_FULL = L - 1
CHUNK = 16


def _register_dve_op(name, spec, subdim=False):
    for op in dve_ops.OPS:
        if op.name == name:
            return op
    opcode = max(dve_ops._SUB_OPCODE_FOR_NAME.values()) + 1
    assert opcode < 0x20
    shas = {}
    for ver in ("v3", "v4"):
        try:
            uops = lower(spec, ver=ver)
            shas[ver] = DveOpSpec(
                name=name, opcode=opcode, uops=uops,
                rd1_en=dve_ops.has_src1(spec),
            ).sha(ver)
        except Exception:
            pass
    op = dve_ops.DveOp(name, spec, subdim=subdim, uops_sha=shas)
    dve_ops.OPS.append(op)
    dve_ops._SUB_OPCODE_FOR_NAME[name] = opcode
    dve_ops.CUSTOM_DVE_SPECS[name] = spec
    return op


AXPY = _register_dve_op(
    "ANT_AXPY",
    Spec(body=Src0 + C0 * Src1,
         reference=lambda in0, in1, c0, c1, c2: in0 + c0 * in1),
)

MASKSEL = _register_dve_op(
    "ANT_MASKSEL",
    Spec(body=select(eq(Src1, C0), Src0, Zero),
         reference=lambda in0, in1, c0, c1, c2: np.where(in1 == c0, in0, 0.0)),
)


def _spline_tables(times, a, b, c, d):
    a = np.asarray(a, np.float64)
    b_ = np.asarray(b, np.float64)
    c_ = np.asarray(c, np.float64)
    d_ = np.asarray(d, np.float64)
    tail = (a[:, -1] + b_[:, -1] + 0.5 * c_[:, -1] + d_[:, -1] / 3.0)[:, None]
    A = np.concatenate([a, tail], axis=1)
    X13 = a + b_ / 3.0 + c_ / 18.0 + d_ / 81.0
    X23 = a + (2.0 / 3.0) * b_ + (2.0 / 9.0) * c_ + (8.0 / 81.0) * d_
    return A, X13, X23


def build_program(T=T_FULL, b1_nonzero=False, repeats=1):
    nc = bacc.Bacc()
    n_chunks = (T + CHUNK - 1) // CHUNK
    t_pad = n_chunks * CHUNK

    cf_in = nc.declare_dram_parameter("cf", [C_HH, t_pad, 4, BC], F32, isOutput=False)
    a0_in = nc.declare_dram_parameter("a0", [C_IN, BC], F32, isOutput=False)
    # slabs [64, C_HH]: W1z, W1z/8, W1z/3, -2/3 W1z, 4/3 W1z, -2 W1z
    wz_in = nc.declare_dram_parameter("wz", [6, C_HID, C_HH], F32, isOutput=False)
    w1x_in = nc.declare_dram_parameter("w1x", [C_IN, C_HH], F32, isOutput=False)
    w2_in = nc.declare_dram_parameter("w2", [C_HH, C_HID], F32, isOutput=False)
    winit_in = nc.declare_dram_parameter("winit", [C_IN, C_HID], F32, isOutput=False)
    wout_in = nc.declare_dram_parameter("wout", [C_HID, C_OUT], F32, isOutput=False)
    bvec_in = nc.declare_dram_parameter("bvec", [4, 128], F32, isOutput=False)
    fi_in = nc.declare_dram_parameter("fi", [C_HID, BC], F32, isOutput=False)
    out_ext = nc.declare_dram_parameter("out", [C_OUT, BC], F32, isOutput=True)

    import contextlib
    with tile.TileContext(nc) as tc, contextlib.ExitStack() as ctx:
        singles = ctx.enter_context(tc.tile_pool(name="singles", bufs=1))
        cf_pool = ctx.enter_context(tc.tile_pool(name="cf", bufs=3))
        hpool = ctx.enter_context(tc.tile_pool(name="hpool", bufs=3))
        kpool = ctx.enter_context(tc.tile_pool(name="kpool", bufs=2))
        zpool = ctx.enter_context(tc.tile_pool(name="zpool", bufs=2))
        gpool = ctx.enter_context(tc.tile_pool(name="gpool", bufs=2))
        p1pool = ctx.enter_context(tc.tile_pool(name="p1", bufs=3, space="PSUM"))
        p2pool = ctx.enter_context(tc.tile_pool(name="p2", bufs=4, space="PSUM"))

        a0t = singles.tile([128, BC], F32)
        nc.sync.dma_start(out=a0t[0:C_IN, :], in_=a0_in[:, :])
        wz = []
        for v in range(6):
            wv = singles.tile([128, C_HH], F32, name=f"wz{v}")
            nc.sync.dma_start(out=wv[0:C_HID, :], in_=wz_in[v, :, :])
            wz.append(wv)
        W_1, W_18, W_13, W_M23, W_43, W_M2 = wz
        w1x = singles.tile([128, C_HH], F32)
        nc.sync.dma_start(out=w1x[0:C_IN, :], in_=w1x_in[:, :])
        w2 = singles.tile([128, C_HID], F32)
        nc.sync.dma_start(out=w2[:, :], in_=w2_in[:, :])
        winit = singles.tile([128, C_HID], F32)
        nc.sync.dma_start(out=winit[0:C_IN, :], in_=winit_in[:, :])
        wout = singles.tile([128, C_OUT], F32)
        nc.sync.dma_start(out=wout[0:64, :], in_=wout_in[:, :])
        bvec = singles.tile([128, 4], F32)
        for r in range(4):
            nc.sync.dma_start(out=bvec[:, r:r + 1],
                              in_=bvec_in[r:r + 1, :].rearrange("o p -> p o"))
        fi_rep = singles.tile([128, BC], F32)
        nc.sync.dma_start(out=fi_rep[0:64, :], in_=fi_in[:, :])

        zT = singles.tile([128, BC], F32)
        nc.vector.memset(zT[0:64, :], 0.0)

        def load_chunk(chk):
            cft = cf_pool.tile([128, CHUNK * 4 * BC], F32, name="cft", tag="cft")
            nc.sync.dma_start(
                out=cft[:, :].rearrange("c (t e b) -> c t e b", t=CHUNK, e=4),
                in_=cf_in[:, chk * CHUNK:(chk + 1) * CHUNK, :, :],
            )
            return cft

        cft = load_chunk(0)

        # ---- z0 ----
        p0 = p1pool.tile([128, BC], F32, name="p1t", tag="p1")
        nc.tensor.matmul(p0[0:64, :], winit[0:C_IN, :], a0t[0:C_IN, :],
                         start=True, stop=True, tile_position=(0, 0))
        z = zpool.tile([128, BC], F32, name="z", tag="z")
        nc.scalar.activation(z[0:64, :], p0[0:64, :], AFT.Identity,
                             bias=bvec[0:64, 2:3])
        g0 = gpool.tile([128, BC], F32, name="g", tag="g")
        nc.vector._custom_dve(MASKSEL, out=g0[0:64, :], in0=z[0:64, :],
                              in1=fi_rep[0:64, :], s0=0.0)
        nc.vector.tensor_add(zT[0:64, :], zT[0:64, :], g0[0:64, :])

        k4_prev = None

        b1bias = bvec[0:128, 0:1]

        # "front half" of a step: open its p1 bank with the early zp2 spine
        # matmul and RMW-add the E0 term on DVE. Pre-emitted during eval 4 of
        # the PREVIOUS step (right after zp2 is computed, before any DVE op
        # that waits on tanh_k4) so both stay off the critical path and the
        # only path op at the step boundary is the W1z/8 @ k4 matmul.
        _loaded_first = [False]

        def front_half(step_idx, zsrc):
            nonlocal cft
            if step_idx % CHUNK == 0 and _loaded_first[0]:
                cft = load_chunk(step_idx // CHUNK)
            _loaded_first[0] = True
            base = (step_idx % CHUNK) * 4 * BC
            p1n = p1pool.tile([128, BC], F32, name="p1t", tag="p1")
            nc.tensor.matmul(p1n[:, :], W_1[0:64, :], zsrc[0:64, :],
                             start=True, stop=False, tile_position=(0, 0))
            nc.vector.tensor_add(p1n[:, :], p1n[:, :],
                                 cft[:, base:base + BC])
            return p1n, cft, base

        p1_state = front_half(0, z)   # step 0: zp2_prev := z0, no k4 term

        for _rep in range(repeats):
            for t in range(T):
                p1, bcft, base = p1_state

                def xs(e, _c=bcft, _b=base):
                    return _c[:, _b + e * BC: _b + (e + 1) * BC]

                def add_E(e):
                    nc.vector.tensor_add(p1[:, :], p1[:, :], xs(e))

                if k4_prev is not None:
                    nc.tensor.matmul(p1[:, :], W_18[0:64, :], k4_prev[0:64, :],
                                     start=False, stop=False, tile_position=(0, 0))

                k = [None] * 4
                h = [None] * 4
                q = [None] * 4

                # ---- eval 1 ----
                h[0] = hpool.tile([128, BC], F32, name="h", tag="h")
                nc.scalar.activation(h[0][:, :], p1[:, :], AFT.Tanh, bias=b1bias)
                q[0] = p2pool.tile([128, BC], F32, name="p2t", tag="p2")
                nc.tensor.matmul(q[0][0:64, :], w2[:, :], h[0][:, :],
                                 start=True, stop=True, tile_position=(0, 0))
                add_E(1)
                k[0] = kpool.tile([128, BC], F32, name="k1", tag="k1")
                nc.scalar.activation(k[0][0:64, :], q[0][0:64, :], AFT.Tanh,
                                     bias=bvec[0:64, 1:2])
                zu = hpool.tile([128, BC], F32, name="zu", tag="zu")
                nc.vector._custom_dve(AXPY, out=zu[0:64, :], in0=z[0:64, :],
                                      in1=k[0][0:64, :], s0=0.125)
                # ---- eval 2 ----
                nc.tensor.matmul(p1[:, :], W_13[0:64, :], k[0][0:64, :],
                                 start=False, stop=False, tile_position=(0, 0))
                h[1] = hpool.tile([128, BC], F32, name="h", tag="h")
                nc.scalar.activation(h[1][:, :], p1[:, :], AFT.Tanh, bias=b1bias)
                q[1] = p2pool.tile([128, BC], F32, name="p2t", tag="p2")
                nc.tensor.matmul(q[1][0:64, :], w2[:, :], h[1][:, :],
                                 start=True, stop=True, tile_position=(0, 0))
                add_E(2)
                nc.tensor.matmul(p1[:, :], W_M23[0:64, :], k[0][0:64, :],
                                 start=False, stop=False, tile_position=(0, 0))
                k[1] = kpool.tile([128, BC], F32, name="k2", tag="k2")
                nc.scalar.activation(k[1][0:64, :], q[1][0:64, :], AFT.Tanh,
                                     bias=bvec[0:64, 1:2])
                zv = hpool.tile([128, BC], F32, name="zv", tag="zv")
                nc.vector._custom_dve(AXPY, out=zv[0:64, :], in0=zu[0:64, :],
                                      in1=k[1][0:64, :], s0=0.375)
                # ---- eval 3 ----
                nc.tensor.matmul(p1[:, :], W_1[0:64, :], k[1][0:64, :],
                                 start=False, stop=False, tile_position=(0, 0))
                h[2] = hpool.tile([128, BC], F32, name="h", tag="h")
                nc.scalar.activation(h[2][:, :], p1[:, :], AFT.Tanh, bias=b1bias)
                q[2] = p2pool.tile([128, BC], F32, name="p2t", tag="p2")
                nc.tensor.matmul(q[2][0:64, :], w2[:, :], h[2][:, :],
                                 start=True, stop=True, tile_position=(0, 0))
                add_E(3)
                nc.tensor.matmul(p1[:, :], W_43[0:64, :], k[0][0:64, :],
                                 start=False, stop=False, tile_position=(0, 0))
                nc.tensor.matmul(p1[:, :], W_M2[0:64, :], k[1][0:64, :],
                                 start=False, stop=False, tile_position=(0, 0))
                k[2] = kpool.tile([128, BC], F32, name="k3", tag="k3")
                nc.scalar.activation(k[2][0:64, :], q[2][0:64, :], AFT.Tanh,
                                     bias=bvec[0:64, 1:2])
                # zp2 = z + (k1+3k2+3k3)/8, built incrementally (zu, zv above)
                zp2 = zpool.tile([128, BC], F32, name="zp2", tag="zp2")
                nc.vector._custom_dve(AXPY, out=zp2[0:64, :], in0=zv[0:64, :],
                                      in1=k[2][0:64, :], s0=0.375)
                # ---- eval 4 ----
                nc.tensor.matmul(p1[:, :], W_1[0:64, :], k[2][0:64, :],
                                 start=False, stop=True, tile_position=(0, 0))
                h[3] = hpool.tile([128, BC], F32, name="h", tag="h")
                nc.scalar.activation(h[3][:, :], p1[:, :], AFT.Tanh, bias=b1bias)
                q[3] = p2pool.tile([128, BC], F32, name="p2t", tag="p2")
                nc.tensor.matmul(q[3][0:64, :], w2[:, :], h[3][:, :],
                                 start=True, stop=True, tile_position=(0, 0))
                # front half issued after the path W2 matmul: keeps the PE
                # in-order queue from stalling W2.h4 behind W_1.zp2 if the
                # DVE zp2 chain runs late.
                if not (_rep == repeats - 1 and t == T - 1):
                    p1_state = front_half(t + 1 if t + 1 < T else 0, zp2)
                k[3] = kpool.tile([128, BC], F32, name="k4", tag="k4")
                nc.scalar.activation(k[3][0:64, :], q[3][0:64, :], AFT.Tanh,
                                     bias=bvec[0:64, 1:2])
                # z' = zp2 + k4/8 ; gather
                znew = zpool.tile([128, BC], F32, name="z", tag="z")
                nc.vector._custom_dve(AXPY, out=znew[0:64, :], in0=zp2[0:64, :],
                                      in1=k[3][0:64, :], s0=0.125)
                g = gpool.tile([128, BC], F32, name="g", tag="g")
                nc.vector._custom_dve(MASKSEL, out=g[0:64, :], in0=znew[0:64, :],
                                      in1=fi_rep[0:64, :], s0=float(t + 1))
                nc.vector.tensor_add(zT[0:64, :], zT[0:64, :], g[0:64, :])

                z = znew
                k4_prev = k[3]

        # ---- readout ----
        po = p2pool.tile([128, BC], F32, name="po", tag="p2")
        nc.tensor.matmul(po[0:C_OUT, :], wout[0:64, :], zT[0:64, :],
                         start=True, stop=True, tile_position=(0, 0))
        ot = singles.tile([128, BC], F32)
        nc.scalar.activation(ot[0:C_OUT, :], po[0:C_OUT, :], AFT.Identity,
                             bias=bvec[0:C_OUT, 3:4])
        nc.sync.dma_start(out=out_ext[:, :], in_=ot[0:C_OUT, :])

    nc.compile()
    return nc


def prepare_inputs(times, coeff_a, coeff_b, coeff_two_c, coeff_three_d,
                   final_index, W_init, b_init, W1, b1, W2, b2, W_out, b_out,
                   T=T_FULL):
    fi = np.asarray(final_index).astype(np.int64)
    W1 = np.asarray(W1, np.float64)
    b1 = np.asarray(b1, np.float32)
    W2_ = np.asarray(W2, np.float32)
    b2_ = np.asarray(b2, np.float32)
    W_init_ = np.asarray(W_init, np.float32)
    b_init_ = np.asarray(b_init, np.float32)
    W_out_ = np.asarray(W_out, np.float32)
    b_out_ = np.asarray(b_out, np.float32)

    A, X13, X23 = _spline_tables(times, coeff_a, coeff_b, coeff_two_c,
                                 coeff_three_d)
    b1_nonzero = bool(np.any(b1 != 0))
    n_chunks = (T + CHUNK - 1) // CHUNK
    t_pad = n_chunks * CHUNK

    At = np.transpose(A, (2, 1, 0))
    X13t = np.transpose(X13, (2, 1, 0))
    X23t = np.transpose(X23, (2, 1, 0))
    Xd = np.zeros((C_IN, t_pad, 4, B), np.float64)
    Xd[:, :T, 0] = At[:, :T]
    Xd[:, :T, 1] = (X13t - At[:, :L - 1])[:, :T]
    Xd[:, :T, 2] = (X23t - X13t)[:, :T]
    Xd[:, :T, 3] = (At[:, 1:] - X23t)[:, :T]
    W1x64 = W1[C_HID:]
    E = np.einsum("cteb,ch->hteb", Xd, W1x64, optimize=True)
    cf_all = np.ascontiguousarray(E, np.float32)
    a0_all = np.ascontiguousarray(At[:, 0], np.float32)

    W1z = W1[:C_HID]
    W1x_ = np.ascontiguousarray(W1[C_HID:].astype(np.float32))

    wz = np.stack([W1z, W1z / 8.0, W1z / 3.0, -2.0 / 3.0 * W1z,
                   4.0 / 3.0 * W1z, -2.0 * W1z]).astype(np.float32)

    bvec = np.zeros((4, 128), np.float32)
    bvec[0, :C_HH] = b1
    bvec[1, :C_HID] = b2_
    bvec[2, :C_HID] = b_init_
    bvec[3, :C_OUT] = b_out_

    in_maps = []
    for core in range(N_CORES):
        cols = slice(core * BC, (core + 1) * BC)
        in_maps.append({
            "cf": np.ascontiguousarray(cf_all[:, :, :, cols]),
            "a0": np.ascontiguousarray(a0_all[:, cols]),
            "wz": wz,
            "w1x": W1x_,
            "w2": W2_,
            "winit": W_init_,
            "wout": W_out_,
            "bvec": bvec,
            "fi": np.ascontiguousarray(
                np.broadcast_to(fi[cols].astype(np.float32), (C_HID, BC))),
        })
    return in_maps, b1_nonzero


_PROGRAM_CACHE = {}


def run(inputs, T=T_FULL, trace=False):
    in_maps, b1_nonzero = prepare_inputs(T=T, **inputs)
    key = (T, b1_nonzero)
    if key not in _PROGRAM_CACHE:
        _PROGRAM_CACHE[key] = build_program(T=T, b1_nonzero=b1_nonzero)
    nc = _PROGRAM_CACHE[key]
    res = run_bass_kernel_spmd(nc, in_maps, core_ids=list(range(N_CORES)),
                               trace=trace)
    outs = [res.results[c]["out"] for c in range(N_CORES)]
    full = np.concatenate([o.T for o in outs], axis=0).astype(np.float32)
    return full, res


def kernel(**inputs):
    out, _ = run(inputs)
    return out



# revision 3
# speedup vs baseline: 1.3071x; 1.0110x over previous
"""Trainium2 Bass kernel for nn_DirectRecurrentODE (v3).

Changes vs v1: zp2 = z + (k1+3k2+3k3)/8 is built incrementally (one AXPY
after each k tanh) so it is ready one DVE op after tanh-k3, and front_half
is issued after the path W2.h4 matmul so the in-order PE queue cannot stall
the path behind the zp2-dependent spine opener. E-adds stay on DVE (the
E-on-PE variant measured much slower: extra per-matmul weight loads
overflow the PE idle windows between path matmuls).

Computation (mirrors the reference):
  X(t): natural cubic spline over per-batch coeffs; f(t,z) = 2-layer tanh MLP
  on [z, X(t)]; rk4 3/8-rule scan over times=arange(512); per-batch
  final_index gather; linear readout.

Mapping (latency-optimized: total time ~= 511 x per-step serial latency):
- Data-parallel over batch: 512 -> 8 cores x 64; one 64-wide chain per core;
  channels on partitions, batch on free dim.
- Per-step critical path is exactly 4 evals x [tanh_k -> W1z-variant matmul
  -> tanh_h -> W2 matmul] (8 matmuls + 8 tanh + 16 semaphore hops). HW
  matmuls cost ~3x the cost model (~300ns marginal, measured), so everything
  else is kept OFF both the path and the PE:
  * spline E-terms (host-precomputed W1x^T X, streamed) enter via DVE: the
    eval-1 term seeds the fresh PSUM bank with a plain DVE copy and the
    spine matmuls accumulate on top with start=False (keeps the seed off the
    critical path); later eval deltas are DVE adds after each tanh_h read.
  * RK4 k-combinations: the one new k-term per eval is a pre-scaled W1z
    matmul on the path; cross-eval terms (k1, k2 reuses) are extra pre-scaled
    matmuls issued right after their k is ready (PE slack).
  * z' update (3/8-rule) and zp2 = z' - k4/8 on DVE (off-path); the next
    step's spine is W1z @ zp2 (early) + (W1z/8) @ k4 (path).
  * final_index gather: DVE mask-select + accumulate into zT each step.
- Host: float64 spline tables, E einsum, weight variant pre-scaling,
  shard/unshard.
"""
import sys
import numpy as np

for _p in ("/opt/trn_rl_repo",):
    if _p not in sys.path:
        sys.path.append(_p)

import concourse.bass as bass
import concourse.bacc as bacc
import concourse.tile as tile
from concourse import mybir
from concourse.bass_utils import run_bass_kernel_spmd
from concourse import dve_ops
from concourse.dve_spec import Spec, Src0, Src1, C0, Zero, eq, select, lower
from concourse.dve_uop import DveOpSpec

F32 = mybir.dt.float32
AFT = mybir.ActivationFunctionType

B, L, C_IN, C_HID, C_HH, C_OUT = 512, 512, 32, 64, 128, 10
N_CORES = 8
BC = B // N_CORES
T_FULL = L - 1
CHUNK = 16


def _register_dve_op(name, spec, subdim=False):
    for op in dve_ops.OPS:
        if op.name == name:
            return op
    opcode = max(dve_ops._SUB_OPCODE_FOR_NAME.values()) + 1
    assert opcode < 0x20
    shas = {}
    for ver in ("v3", "v4"):
        try:
            uops = lower(spec, ver=ver)
            shas[ver] = DveOpSpec(
                name=name, opcode=opcode, uops=uops,
                rd1_en=dve_ops.has_src1(spec),
            ).sha(ver)
        except Exception:
            pass
    op = dve_ops.DveOp(name, spec, subdim=subdim, uops_sha=shas)
    dve_ops.OPS.append(op)
    dve_ops._SUB_OPCODE_FOR_NAME[name] = opcode
    dve_ops.CUSTOM_DVE_SPECS[name] = spec
    return op


AXPY = _register_dve_op(
    "ANT_AXPY",
    Spec(body=Src0 + C0 * Src1,
         reference=lambda in0, in1, c0, c1, c2: in0 + c0 * in1),
)

MASKSEL = _register_dve_op(
    "ANT_MASKSEL",
    Spec(body=select(eq(Src1, C0), Src0, Zero),
         reference=lambda in0, in1, c0, c1, c2: np.where(in1 == c0, in0, 0.0)),
)


def _spline_tables(times, a, b, c, d):
    a = np.asarray(a, np.float64)
    b_ = np.asarray(b, np.float64)
    c_ = np.asarray(c, np.float64)
    d_ = np.asarray(d, np.float64)
    tail = (a[:, -1] + b_[:, -1] + 0.5 * c_[:, -1] + d_[:, -1] / 3.0)[:, None]
    A = np.concatenate([a, tail], axis=1)
    X13 = a + b_ / 3.0 + c_ / 18.0 + d_ / 81.0
    X23 = a + (2.0 / 3.0) * b_ + (2.0 / 9.0) * c_ + (8.0 / 81.0) * d_
    return A, X13, X23


def _prune_sync(nc):
    """Remove provably redundant semaphore waits (post tile sem-assignment,
    pre compile).

    TRN2 instructions encode one wait; extras become EventSemaphore
    instructions costing sequencer dispatch on the consumer's queue (the
    critical-path engines here). Sound prunes, using only per-semaphore
    monotonicity and serial in-order engine queues:
      R2: waits on the consumer's own engine semaphore (in-order queue).
      R1: waits dominated by an earlier same-queue wait on the same sem.
      R3: waits implied transitively via kept waits' producer clocks.
    DMA-queue sems never contribute queue-order coverage (transfers overlap
    in flight); they participate only via same-sem dominance and explicit
    wait clocks, which avoids the cross-queue HWDGE unsoundness that got
    the stock optimize_sems pass disabled.
    """
    import bisect

    SERIAL = {"EngineType.PE", "EngineType.Activation", "EngineType.DVE",
              "EngineType.Pool"}
    f = nc.m.functions[0]
    insts = [i for blk in f.blocks for i in blk.instructions]

    # sem -> sole updating engine (None if multiple/none)
    upd_eng = {}
    # Monotonicity: dominance reasoning is only sound for sems that are
    # never decremented/rewritten while still being waited on (barrier sems
    # are gathered then subtracted; engine sems are cleared only in the
    # epilogue after their last wait, which is behind an all-engine barrier).
    last_wait_pos = {}
    first_nonmono_pos = {}
    for pos, inst in enumerate(insts):
        si = inst.sync_info
        if si is None:
            continue
        for w in (si.on_wait or []):
            last_wait_pos[w.ant_name] = pos
        for u in (si.on_update or []):
            if u.update_mode != "sem-inc" and u.ant_name not in first_nonmono_pos:
                first_nonmono_pos[u.ant_name] = pos
    usable = lambda S: first_nonmono_pos.get(S, 1 << 60) > last_wait_pos.get(S, -1)
    for inst in insts:
        si = inst.sync_info
        if si is None:
            continue
        for u in (si.on_update or []):
            e = str(inst.engine)
            if u.ant_name not in upd_eng:
                upd_eng[u.ant_name] = e
            elif upd_eng[u.ant_name] != e:
                upd_eng[u.ant_name] = None
    own_sem = {}  # engine -> its serial engine sem
    for s, e in upd_eng.items():
        if e in SERIAL and s.startswith(e.split(".")[-1] + "_"):
            own_sem[e] = s

    sem_val = {}          # sem -> current cumulative value in sweep order
    prod = {}             # sem -> (values list, clocks list) at >= thresholds
    comp = {}             # engine -> completion clock of prev instr on queue
    n_rm = n_tot = 0

    def join(a, b):
        for k, v in b.items():
            if a.get(k, 0) < v:
                a[k] = v

    for inst in insts:
        si = inst.sync_info
        e = str(inst.engine)
        C = {}
        if e in SERIAL and e in comp:
            join(C, comp[e])
        if si is not None:
            waits = list(si.on_wait or [])
            n_tot += len(waits)
            bad = any(w.wait_mode != "sem-ge-imm" or w.wait_reg is not None
                      for w in waits)
            if bad:
                for w in waits:
                    if usable(w.ant_name):
                        C[w.ant_name] = max(C.get(w.ant_name, 0),
                                            w.wait_value or 0)
            else:
                kept = []
                for w in sorted(waits, key=lambda w: -(w.wait_value or 0)):
                    S, v = w.ant_name, w.wait_value or 0
                    if not usable(S):
                        kept.append(w)
                        continue
                    if own_sem.get(e) == S:
                        n_rm += 1
                        continue
                    if C.get(S, 0) >= v:
                        n_rm += 1
                        continue
                    kept.append(w)
                    C[S] = max(C.get(S, 0), v)
                    if S in prod:
                        vals, clks = prod[S]
                        j = bisect.bisect_left(vals, v)
                        if j < len(vals):
                            join(C, clks[j])
                if len(kept) != len(waits):
                    kept.sort(key=lambda w: waits.index(w))
                    inst.sync_info = mybir.SyncInfo(
                        on_wait=kept, on_update=list(si.on_update or []))
        # record updates: producers' clocks
        if si is not None:
            for u in (si.on_update or []):
                S = u.ant_name
                if u.update_mode != "sem-inc" or not usable(S):
                    continue
                nv = sem_val.get(S, 0) + (u.update_value or 0)
                sem_val[S] = nv
                pc = dict(C)
                if own_sem.get(e) == S:
                    pc[S] = max(pc.get(S, 0), nv)
                vals, clks = prod.setdefault(S, ([], []))
                vals.append(nv)
                clks.append(pc)
        if e in SERIAL:
            cc = dict(C)
            s = own_sem.get(e)
            if s is not None and usable(s):
                cc[s] = max(cc.get(s, 0), sem_val.get(s, 0))
            comp[e] = cc
    return n_rm, n_tot


def build_program(T=T_FULL, b1_nonzero=False, repeats=1, prune=True):
    nc = bacc.Bacc()
    n_chunks = (T + CHUNK - 1) // CHUNK
    t_pad = n_chunks * CHUNK

    cf_in = nc.declare_dram_parameter("cf", [C_HH, t_pad, 4, BC], F32, isOutput=False)
    a0_in = nc.declare_dram_parameter("a0", [C_IN, BC], F32, isOutput=False)
    # slabs [64, C_HH]: W1z, W1z/8, W1z/3, -2/3 W1z, 4/3 W1z, -2 W1z
    wz_in = nc.declare_dram_parameter("wz", [6, C_HID, C_HH], F32, isOutput=False)
    w1x_in = nc.declare_dram_parameter("w1x", [C_IN, C_HH], F32, isOutput=False)
    w2_in = nc.declare_dram_parameter("w2", [C_HH, C_HID], F32, isOutput=False)
    winit_in = nc.declare_dram_parameter("winit", [C_IN, C_HID], F32, isOutput=False)
    wout_in = nc.declare_dram_parameter("wout", [C_HID, C_OUT], F32, isOutput=False)
    bvec_in = nc.declare_dram_parameter("bvec", [4, 128], F32, isOutput=False)
    fi_in = nc.declare_dram_parameter("fi", [C_HID, BC], F32, isOutput=False)
    out_ext = nc.declare_dram_parameter("out", [C_OUT, BC], F32, isOutput=True)

    import contextlib
    with tile.TileContext(nc) as tc, contextlib.ExitStack() as ctx:
        singles = ctx.enter_context(tc.tile_pool(name="singles", bufs=1))
        cf_pool = ctx.enter_context(tc.tile_pool(name="cf", bufs=3))
        hpool = ctx.enter_context(tc.tile_pool(name="hpool", bufs=3))
        kpool = ctx.enter_context(tc.tile_pool(name="kpool", bufs=2))
        zpool = ctx.enter_context(tc.tile_pool(name="zpool", bufs=2))
        gpool = ctx.enter_context(tc.tile_pool(name="gpool", bufs=2))
        p1pool = ctx.enter_context(tc.tile_pool(name="p1", bufs=3, space="PSUM"))
        p2pool = ctx.enter_context(tc.tile_pool(name="p2", bufs=4, space="PSUM"))

        a0t = singles.tile([128, BC], F32)
        nc.sync.dma_start(out=a0t[0:C_IN, :], in_=a0_in[:, :])
        wz = []
        for v in range(6):
            wv = singles.tile([128, C_HH], F32, name=f"wz{v}")
            nc.sync.dma_start(out=wv[0:C_HID, :], in_=wz_in[v, :, :])
            wz.append(wv)
        W_1, W_18, W_13, W_M23, W_43, W_M2 = wz
        w1x = singles.tile([128, C_HH], F32)
        nc.sync.dma_start(out=w1x[0:C_IN, :], in_=w1x_in[:, :])
        w2 = singles.tile([128, C_HID], F32)
        nc.sync.dma_start(out=w2[:, :], in_=w2_in[:, :])
        winit = singles.tile([128, C_HID], F32)
        nc.sync.dma_start(out=winit[0:C_IN, :], in_=winit_in[:, :])
        wout = singles.tile([128, C_OUT], F32)
        nc.sync.dma_start(out=wout[0:64, :], in_=wout_in[:, :])
        bvec = singles.tile([128, 4], F32)
        for r in range(4):
            nc.sync.dma_start(out=bvec[:, r:r + 1],
                              in_=bvec_in[r:r + 1, :].rearrange("o p -> p o"))
        fi_rep = singles.tile([128, BC], F32)
        nc.sync.dma_start(out=fi_rep[0:64, :], in_=fi_in[:, :])

        zT = singles.tile([128, BC], F32)
        nc.vector.memset(zT[0:64, :], 0.0)

        def load_chunk(chk):
            cft = cf_pool.tile([128, CHUNK * 4 * BC], F32, name="cft", tag="cft")
            nc.sync.dma_start(
                out=cft[:, :].rearrange("c (t e b) -> c t e b", t=CHUNK, e=4),
                in_=cf_in[:, chk * CHUNK:(chk + 1) * CHUNK, :, :],
            )
            return cft

        cft = load_chunk(0)

        # ---- z0 ----
        p0 = p1pool.tile([128, BC], F32, name="p1t", tag="p1")
        nc.tensor.matmul(p0[0:64, :], winit[0:C_IN, :], a0t[0:C_IN, :],
                         start=True, stop=True, tile_position=(0, 0))
        z = zpool.tile([128, BC], F32, name="z", tag="z")
        nc.scalar.activation(z[0:64, :], p0[0:64, :], AFT.Identity,
                             bias=bvec[0:64, 2:3])
        g0 = gpool.tile([128, BC], F32, name="g", tag="g")
        nc.vector._custom_dve(MASKSEL, out=g0[0:64, :], in0=z[0:64, :],
                              in1=fi_rep[0:64, :], s0=0.0)
        nc.vector.tensor_add(zT[0:64, :], zT[0:64, :], g0[0:64, :])

        k4_prev = None

        b1bias = bvec[0:128, 0:1]

        # "front half" of a step: open its p1 bank with the early zp2 spine
        # matmul and RMW-add the E0 term on DVE. Pre-emitted during eval 4 of
        # the PREVIOUS step (right after zp2 is computed, before any DVE op
        # that waits on tanh_k4) so both stay off the critical path and the
        # only path op at the step boundary is the W1z/8 @ k4 matmul.
        _loaded_first = [False]

        def front_half(step_idx, zsrc):
            nonlocal cft
            if step_idx % CHUNK == 0 and _loaded_first[0]:
                cft = load_chunk(step_idx // CHUNK)
            _loaded_first[0] = True
            base = (step_idx % CHUNK) * 4 * BC
            p1n = p1pool.tile([128, BC], F32, name="p1t", tag="p1")
            nc.tensor.matmul(p1n[:, :], W_1[0:64, :], zsrc[0:64, :],
                             start=True, stop=False, tile_position=(0, 0))
            nc.vector.tensor_add(p1n[:, :], p1n[:, :],
                                 cft[:, base:base + BC])
            return p1n, cft, base

        p1_state = front_half(0, z)   # step 0: zp2_prev := z0, no k4 term

        for _rep in range(repeats):
            for t in range(T):
                p1, bcft, base = p1_state

                def xs(e, _c=bcft, _b=base):
                    return _c[:, _b + e * BC: _b + (e + 1) * BC]

                def add_E(e):
                    nc.vector.tensor_add(p1[:, :], p1[:, :], xs(e))

                if k4_prev is not None:
                    nc.tensor.matmul(p1[:, :], W_18[0:64, :], k4_prev[0:64, :],
                                     start=False, stop=False, tile_position=(0, 0))

                k = [None] * 4
                h = [None] * 4
                q = [None] * 4

                # ---- eval 1 ----
                h[0] = hpool.tile([128, BC], F32, name="h", tag="h")
                nc.scalar.activation(h[0][:, :], p1[:, :], AFT.Tanh, bias=b1bias)
                q[0] = p2pool.tile([128, BC], F32, name="p2t", tag="p2")
                nc.tensor.matmul(q[0][0:64, :], w2[:, :], h[0][:, :],
                                 start=True, stop=True, tile_position=(0, 0))
                add_E(1)
                k[0] = kpool.tile([128, BC], F32, name="k1", tag="k1")
                nc.scalar.activation(k[0][0:64, :], q[0][0:64, :], AFT.Tanh,
                                     bias=bvec[0:64, 1:2])
                zu = hpool.tile([128, BC], F32, name="zu", tag="zu")
                nc.vector._custom_dve(AXPY, out=zu[0:64, :], in0=z[0:64, :],
                                      in1=k[0][0:64, :], s0=0.125)
                # ---- eval 2 ----
                nc.tensor.matmul(p1[:, :], W_13[0:64, :], k[0][0:64, :],
                                 start=False, stop=False, tile_position=(0, 0))
                h[1] = hpool.tile([128, BC], F32, name="h", tag="h")
                nc.scalar.activation(h[1][:, :], p1[:, :], AFT.Tanh, bias=b1bias)
                q[1] = p2pool.tile([128, BC], F32, name="p2t", tag="p2")
                nc.tensor.matmul(q[1][0:64, :], w2[:, :], h[1][:, :],
                                 start=True, stop=True, tile_position=(0, 0))
                add_E(2)
                nc.tensor.matmul(p1[:, :], W_M23[0:64, :], k[0][0:64, :],
                                 start=False, stop=False, tile_position=(0, 0))
                k[1] = kpool.tile([128, BC], F32, name="k2", tag="k2")
                nc.scalar.activation(k[1][0:64, :], q[1][0:64, :], AFT.Tanh,
                                     bias=bvec[0:64, 1:2])
                zv = hpool.tile([128, BC], F32, name="zv", tag="zv")
                nc.vector._custom_dve(AXPY, out=zv[0:64, :], in0=zu[0:64, :],
                                      in1=k[1][0:64, :], s0=0.375)
                # ---- eval 3 ----
                nc.tensor.matmul(p1[:, :], W_1[0:64, :], k[1][0:64, :],
                                 start=False, stop=False, tile_position=(0, 0))
                h[2] = hpool.tile([128, BC], F32, name="h", tag="h")
                nc.scalar.activation(h[2][:, :], p1[:, :], AFT.Tanh, bias=b1bias)
                q[2] = p2pool.tile([128, BC], F32, name="p2t", tag="p2")
                nc.tensor.matmul(q[2][0:64, :], w2[:, :], h[2][:, :],
                                 start=True, stop=True, tile_position=(0, 0))
                add_E(3)
                nc.tensor.matmul(p1[:, :], W_43[0:64, :], k[0][0:64, :],
                                 start=False, stop=False, tile_position=(0, 0))
                nc.tensor.matmul(p1[:, :], W_M2[0:64, :], k[1][0:64, :],
                                 start=False, stop=False, tile_position=(0, 0))
                k[2] = kpool.tile([128, BC], F32, name="k3", tag="k3")
                nc.scalar.activation(k[2][0:64, :], q[2][0:64, :], AFT.Tanh,
                                     bias=bvec[0:64, 1:2])
                # zp2 = z + (k1+3k2+3k3)/8, built incrementally (zu, zv above)
                zp2 = zpool.tile([128, BC], F32, name="zp2", tag="zp2")
                nc.vector._custom_dve(AXPY, out=zp2[0:64, :], in0=zv[0:64, :],
                                      in1=k[2][0:64, :], s0=0.375)
                # ---- eval 4 ----
                nc.tensor.matmul(p1[:, :], W_1[0:64, :], k[2][0:64, :],
                                 start=False, stop=True, tile_position=(0, 0))
                h[3] = hpool.tile([128, BC], F32, name="h", tag="h")
                nc.scalar.activation(h[3][:, :], p1[:, :], AFT.Tanh, bias=b1bias)
                q[3] = p2pool.tile([128, BC], F32, name="p2t", tag="p2")
                nc.tensor.matmul(q[3][0:64, :], w2[:, :], h[3][:, :],
                                 start=True, stop=True, tile_position=(0, 0))
                # front half issued after the path W2 matmul: keeps the PE
                # in-order queue from stalling W2.h4 behind W_1.zp2 if the
                # DVE zp2 chain runs late.
                if not (_rep == repeats - 1 and t == T - 1):
                    p1_state = front_half(t + 1 if t + 1 < T else 0, zp2)
                k[3] = kpool.tile([128, BC], F32, name="k4", tag="k4")
                nc.scalar.activation(k[3][0:64, :], q[3][0:64, :], AFT.Tanh,
                                     bias=bvec[0:64, 1:2])
                # z' = zp2 + k4/8 ; gather
                znew = zpool.tile([128, BC], F32, name="z", tag="z")
                nc.vector._custom_dve(AXPY, out=znew[0:64, :], in0=zp2[0:64, :],
                                      in1=k[3][0:64, :], s0=0.125)
                g = gpool.tile([128, BC], F32, name="g", tag="g")
                nc.vector._custom_dve(MASKSEL, out=g[0:64, :], in0=znew[0:64, :],
                                      in1=fi_rep[0:64, :], s0=float(t + 1))
                nc.vector.tensor_add(zT[0:64, :], zT[0:64, :], g[0:64, :])

                z = znew
                k4_prev = k[3]

        # ---- readout ----
        po = p2pool.tile([128, BC], F32, name="po", tag="p2")
        nc.tensor.matmul(po[0:C_OUT, :], wout[0:64, :], zT[0:64, :],
                         start=True, stop=True, tile_position=(0, 0))
        ot = singles.tile([128, BC], F32)
        nc.scalar.activation(ot[0:C_OUT, :], po[0:C_OUT, :], AFT.Identity,
                             bias=bvec[0:C_OUT, 3:4])
        nc.sync.dma_start(out=out_ext[:, :], in_=ot[0:C_OUT, :])

    if prune:
        _prune_sync(nc)
    nc.compile()
    return nc


def prepare_inputs(times, coeff_a, coeff_b, coeff_two_c, coeff_three_d,
                   final_index, W_init, b_init, W1, b1, W2, b2, W_out, b_out,
                   T=T_FULL):
    fi = np.asarray(final_index).astype(np.int64)
    W1 = np.asarray(W1, np.float64)
    b1 = np.asarray(b1, np.float32)
    W2_ = np.asarray(W2, np.float32)
    b2_ = np.asarray(b2, np.float32)
    W_init_ = np.asarray(W_init, np.float32)
    b_init_ = np.asarray(b_init, np.float32)
    W_out_ = np.asarray(W_out, np.float32)
    b_out_ = np.asarray(b_out, np.float32)

    A, X13, X23 = _spline_tables(times, coeff_a, coeff_b, coeff_two_c,
                                 coeff_three_d)
    b1_nonzero = bool(np.any(b1 != 0))
    n_chunks = (T + CHUNK - 1) // CHUNK
    t_pad = n_chunks * CHUNK

    At = np.transpose(A, (2, 1, 0))
    X13t = np.transpose(X13, (2, 1, 0))
    X23t = np.transpose(X23, (2, 1, 0))
    Xd = np.zeros((C_IN, t_pad, 4, B), np.float64)
    Xd[:, :T, 0] = At[:, :T]
    Xd[:, :T, 1] = (X13t - At[:, :L - 1])[:, :T]
    Xd[:, :T, 2] = (X23t - X13t)[:, :T]
    Xd[:, :T, 3] = (At[:, 1:] - X23t)[:, :T]
    W1x64 = W1[C_HID:]
    E = np.einsum("cteb,ch->hteb", Xd, W1x64, optimize=True)
    cf_all = np.ascontiguousarray(E, np.float32)
    a0_all = np.ascontiguousarray(At[:, 0], np.float32)

    W1z = W1[:C_HID]
    W1x_ = np.ascontiguousarray(W1[C_HID:].astype(np.float32))

    wz = np.stack([W1z, W1z / 8.0, W1z / 3.0, -2.0 / 3.0 * W1z,
                   4.0 / 3.0 * W1z, -2.0 * W1z]).astype(np.float32)

    bvec = np.zeros((4, 128), np.float32)
    bvec[0, :C_HH] = b1
    bvec[1, :C_HID] = b2_
    bvec[2, :C_HID] = b_init_
    bvec[3, :C_OUT] = b_out_

    in_maps = []
    for core in range(N_CORES):
        cols = slice(core * BC, (core + 1) * BC)
        in_maps.append({
            "cf": np.ascontiguousarray(cf_all[:, :, :, cols]),
            "a0": np.ascontiguousarray(a0_all[:, cols]),
            "wz": wz,
            "w1x": W1x_,
            "w2": W2_,
            "winit": W_init_,
            "wout": W_out_,
            "bvec": bvec,
            "fi": np.ascontiguousarray(
                np.broadcast_to(fi[cols].astype(np.float32), (C_HID, BC))),
        })
    return in_maps, b1_nonzero


_PROGRAM_CACHE = {}


def run(inputs, T=T_FULL, trace=False):
    in_maps, b1_nonzero = prepare_inputs(T=T, **inputs)
    key = (T, b1_nonzero)
    if key not in _PROGRAM_CACHE:
        _PROGRAM_CACHE[key] = build_program(T=T, b1_nonzero=b1_nonzero)
    nc = _PROGRAM_CACHE[key]
    res = run_bass_kernel_spmd(nc, in_maps, core_ids=list(range(N_CORES)),
                               trace=trace)
    outs = [res.results[c]["out"] for c in range(N_CORES)]
    full = np.concatenate([o.T for o in outs], axis=0).astype(np.float32)
    return full, res


def kernel(**inputs):
    out, _ = run(inputs)
    return out



# revision 4
# speedup vs baseline: 1.3096x; 1.0020x over previous
"""Trainium2 Bass kernel for nn_DirectRecurrentODE (v4).

Changes vs the original baseline:
- zp2 = z + (k1+3k2+3k3)/8 is built incrementally (one AXPY after each k
  tanh) so it is ready one DVE op after tanh-k3, and front_half is issued
  after the path W2.h4 matmul so the in-order PE queue cannot stall the
  path behind the zp2-dependent spine opener (this removed a reproducible
  ~2.5us/step stall at full T=511).
- _prune_sync removes provably redundant semaphore waits before compile
  (own-queue waits, same-sem-dominated waits, transitively implied waits;
  non-monotone sems like barriers excluded). This eliminates ~10
  EventSemaphore instructions per step from the ACT/DVE sequencer queues
  (~6% measured).
- E-adds stay on DVE (an E-on-PE variant measured much slower: extra
  per-matmul weight loads overflow the PE idle windows between path
  matmuls).

Computation (mirrors the reference):
  X(t): natural cubic spline over per-batch coeffs; f(t,z) = 2-layer tanh MLP
  on [z, X(t)]; rk4 3/8-rule scan over times=arange(512); per-batch
  final_index gather; linear readout.

Mapping (latency-optimized: total time ~= 511 x per-step serial latency):
- Data-parallel over batch: 512 -> 8 cores x 64; one 64-wide chain per core;
  channels on partitions, batch on free dim.
- Per-step critical path is exactly 4 evals x [tanh_k -> W1z-variant matmul
  -> tanh_h -> W2 matmul] (8 matmuls + 8 tanh + 16 semaphore hops). HW
  matmuls cost ~3x the cost model (~300ns marginal, measured), so everything
  else is kept OFF both the path and the PE:
  * spline E-terms (host-precomputed W1x^T X, streamed) enter via DVE: the
    eval-1 term seeds the fresh PSUM bank with a plain DVE copy and the
    spine matmuls accumulate on top with start=False (keeps the seed off the
    critical path); later eval deltas are DVE adds after each tanh_h read.
  * RK4 k-combinations: the one new k-term per eval is a pre-scaled W1z
    matmul on the path; cross-eval terms (k1, k2 reuses) are extra pre-scaled
    matmuls issued right after their k is ready (PE slack).
  * z' update (3/8-rule) and zp2 = z' - k4/8 on DVE (off-path); the next
    step's spine is W1z @ zp2 (early) + (W1z/8) @ k4 (path).
  * final_index gather: DVE mask-select + accumulate into zT each step.
- Host: float64 spline tables, E einsum, weight variant pre-scaling,
  shard/unshard.
"""
import sys
import numpy as np

for _p in ("/opt/trn_rl_repo",):
    if _p not in sys.path:
        sys.path.append(_p)

import concourse.bass as bass
import concourse.bacc as bacc
import concourse.tile as tile
from concourse import mybir
from concourse.bass_utils import run_bass_kernel_spmd
from concourse import dve_ops
from concourse.dve_spec import Spec, Src0, Src1, C0, Zero, eq, select, lower
from concourse.dve_uop import DveOpSpec

F32 = mybir.dt.float32
AFT = mybir.ActivationFunctionType

B, L, C_IN, C_HID, C_HH, C_OUT = 512, 512, 32, 64, 128, 10
N_CORES = 8
BC = B // N_CORES
T_FULL = L - 1
CHUNK = 16


def _register_dve_op(name, spec, subdim=False):
    for op in dve_ops.OPS:
        if op.name == name:
            return op
    opcode = max(dve_ops._SUB_OPCODE_FOR_NAME.values()) + 1
    assert opcode < 0x20
    shas = {}
    for ver in ("v3", "v4"):
        try:
            uops = lower(spec, ver=ver)
            shas[ver] = DveOpSpec(
                name=name, opcode=opcode, uops=uops,
                rd1_en=dve_ops.has_src1(spec),
            ).sha(ver)
        except Exception:
            pass
    op = dve_ops.DveOp(name, spec, subdim=subdim, uops_sha=shas)
    dve_ops.OPS.append(op)
    dve_ops._SUB_OPCODE_FOR_NAME[name] = opcode
    dve_ops.CUSTOM_DVE_SPECS[name] = spec
    return op


AXPY = _register_dve_op(
    "ANT_AXPY",
    Spec(body=Src0 + C0 * Src1,
         reference=lambda in0, in1, c0, c1, c2: in0 + c0 * in1),
)

MASKSEL = _register_dve_op(
    "ANT_MASKSEL",
    Spec(body=select(eq(Src1, C0), Src0, Zero),
         reference=lambda in0, in1, c0, c1, c2: np.where(in1 == c0, in0, 0.0)),
)


def _spline_tables(times, a, b, c, d):
    a = np.asarray(a, np.float64)
    b_ = np.asarray(b, np.float64)
    c_ = np.asarray(c, np.float64)
    d_ = np.asarray(d, np.float64)
    tail = (a[:, -1] + b_[:, -1] + 0.5 * c_[:, -1] + d_[:, -1] / 3.0)[:, None]
    A = np.concatenate([a, tail], axis=1)
    X13 = a + b_ / 3.0 + c_ / 18.0 + d_ / 81.0
    X23 = a + (2.0 / 3.0) * b_ + (2.0 / 9.0) * c_ + (8.0 / 81.0) * d_
    return A, X13, X23


def _prune_sync(nc):
    """Remove provably redundant semaphore waits (post tile sem-assignment,
    pre compile).

    TRN2 instructions encode one wait; extras become EventSemaphore
    instructions costing sequencer dispatch on the consumer's queue (the
    critical-path engines here). Sound prunes, using only per-semaphore
    monotonicity and serial in-order engine queues:
      R2: waits on the consumer's own engine semaphore (in-order queue).
      R1: waits dominated by an earlier same-queue wait on the same sem.
      R3: waits implied transitively via kept waits' producer clocks.
    DMA-queue sems never contribute queue-order coverage (transfers overlap
    in flight); they participate only via same-sem dominance and explicit
    wait clocks, which avoids the cross-queue HWDGE unsoundness that got
    the stock optimize_sems pass disabled.
    """
    import bisect

    SERIAL = {"EngineType.PE", "EngineType.Activation", "EngineType.DVE",
              "EngineType.Pool"}
    f = nc.m.functions[0]
    insts = [i for blk in f.blocks for i in blk.instructions]

    # sem -> sole updating engine (None if multiple/none)
    upd_eng = {}
    # Monotonicity: dominance reasoning is only sound for sems that are
    # never decremented/rewritten while still being waited on (barrier sems
    # are gathered then subtracted; engine sems are cleared only in the
    # epilogue after their last wait, which is behind an all-engine barrier).
    last_wait_pos = {}
    first_nonmono_pos = {}
    for pos, inst in enumerate(insts):
        si = inst.sync_info
        if si is None:
            continue
        for w in (si.on_wait or []):
            last_wait_pos[w.ant_name] = pos
        for u in (si.on_update or []):
            if u.update_mode != "sem-inc" and u.ant_name not in first_nonmono_pos:
                first_nonmono_pos[u.ant_name] = pos
    usable = lambda S: first_nonmono_pos.get(S, 1 << 60) > last_wait_pos.get(S, -1)
    for inst in insts:
        si = inst.sync_info
        if si is None:
            continue
        for u in (si.on_update or []):
            e = str(inst.engine)
            if u.ant_name not in upd_eng:
                upd_eng[u.ant_name] = e
            elif upd_eng[u.ant_name] != e:
                upd_eng[u.ant_name] = None
    own_sem = {}  # engine -> its serial engine sem
    for s, e in upd_eng.items():
        if e in SERIAL and s.startswith(e.split(".")[-1] + "_"):
            own_sem[e] = s

    sem_val = {}          # sem -> current cumulative value in sweep order
    prod = {}             # sem -> (values list, clocks list) at >= thresholds
    comp = {}             # engine -> completion clock of prev instr on queue
    n_rm = n_tot = 0

    def join(a, b):
        for k, v in b.items():
            if a.get(k, 0) < v:
                a[k] = v

    for inst in insts:
        si = inst.sync_info
        e = str(inst.engine)
        C = {}
        if e in SERIAL and e in comp:
            join(C, comp[e])
        if si is not None:
            waits = list(si.on_wait or [])
            n_tot += len(waits)
            bad = any(w.wait_mode != "sem-ge-imm" or w.wait_reg is not None
                      for w in waits)
            if bad:
                for w in waits:
                    if usable(w.ant_name):
                        C[w.ant_name] = max(C.get(w.ant_name, 0),
                                            w.wait_value or 0)
            else:
                kept = []
                for w in sorted(waits, key=lambda w: -(w.wait_value or 0)):
                    S, v = w.ant_name, w.wait_value or 0
                    if not usable(S):
                        kept.append(w)
                        continue
                    if own_sem.get(e) == S:
                        n_rm += 1
                        continue
                    if C.get(S, 0) >= v:
                        n_rm += 1
                        continue
                    kept.append(w)
                    C[S] = max(C.get(S, 0), v)
                    if S in prod:
                        vals, clks = prod[S]
                        j = bisect.bisect_left(vals, v)
                        if j < len(vals):
                            join(C, clks[j])
                if len(kept) != len(waits):
                    kept.sort(key=lambda w: waits.index(w))
                    inst.sync_info = mybir.SyncInfo(
                        on_wait=kept, on_update=list(si.on_update or []))
        # record updates: producers' clocks
        if si is not None:
            for u in (si.on_update or []):
                S = u.ant_name
                if u.update_mode != "sem-inc" or not usable(S):
                    continue
                nv = sem_val.get(S, 0) + (u.update_value or 0)
                sem_val[S] = nv
                pc = dict(C)
                if own_sem.get(e) == S:
                    pc[S] = max(pc.get(S, 0), nv)
                vals, clks = prod.setdefault(S, ([], []))
                vals.append(nv)
                clks.append(pc)
        if e in SERIAL:
            cc = dict(C)
            s = own_sem.get(e)
            if s is not None and usable(s):
                cc[s] = max(cc.get(s, 0), sem_val.get(s, 0))
            comp[e] = cc
    return n_rm, n_tot


def build_program(T=T_FULL, b1_nonzero=False, repeats=1, prune=True):
    nc = bacc.Bacc()
    n_chunks = (T + CHUNK - 1) // CHUNK
    t_pad = n_chunks * CHUNK

    cf_in = nc.declare_dram_parameter("cf", [C_HH, t_pad, 4, BC], F32, isOutput=False)
    a0_in = nc.declare_dram_parameter("a0", [C_IN, BC], F32, isOutput=False)
    # slabs [64, C_HH]: W1z, W1z/8, W1z/3, -2/3 W1z, 4/3 W1z, -2 W1z
    wz_in = nc.declare_dram_parameter("wz", [6, C_HID, C_HH], F32, isOutput=False)
    w1x_in = nc.declare_dram_parameter("w1x", [C_IN, C_HH], F32, isOutput=False)
    w2_in = nc.declare_dram_parameter("w2", [C_HH, C_HID], F32, isOutput=False)
    winit_in = nc.declare_dram_parameter("winit", [C_IN, C_HID], F32, isOutput=False)
    wout_in = nc.declare_dram_parameter("wout", [C_HID, C_OUT], F32, isOutput=False)
    bvec_in = nc.declare_dram_parameter("bvec", [4, 128], F32, isOutput=False)
    fi_in = nc.declare_dram_parameter("fi", [C_HID, BC], F32, isOutput=False)
    out_ext = nc.declare_dram_parameter("out", [C_OUT, BC], F32, isOutput=True)

    import contextlib
    with tile.TileContext(nc) as tc, contextlib.ExitStack() as ctx:
        singles = ctx.enter_context(tc.tile_pool(name="singles", bufs=1))
        cf_pool = ctx.enter_context(tc.tile_pool(name="cf", bufs=3))
        hpool = ctx.enter_context(tc.tile_pool(name="hpool", bufs=3))
        kpool = ctx.enter_context(tc.tile_pool(name="kpool", bufs=2))
        zpool = ctx.enter_context(tc.tile_pool(name="zpool", bufs=2))
        gpool = ctx.enter_context(tc.tile_pool(name="gpool", bufs=2))
        p1pool = ctx.enter_context(tc.tile_pool(name="p1", bufs=3, space="PSUM"))
        p2pool = ctx.enter_context(tc.tile_pool(name="p2", bufs=4, space="PSUM"))

        a0t = singles.tile([128, BC], F32)
        nc.sync.dma_start(out=a0t[0:C_IN, :], in_=a0_in[:, :])
        wz = []
        for v in range(6):
            wv = singles.tile([128, C_HH], F32, name=f"wz{v}")
            nc.sync.dma_start(out=wv[0:C_HID, :], in_=wz_in[v, :, :])
            wz.append(wv)
        W_1, W_18, W_13, W_M23, W_43, W_M2 = wz
        w1x = singles.tile([128, C_HH], F32)
        nc.sync.dma_start(out=w1x[0:C_IN, :], in_=w1x_in[:, :])
        w2 = singles.tile([128, C_HID], F32)
        nc.sync.dma_start(out=w2[:, :], in_=w2_in[:, :])
        winit = singles.tile([128, C_HID], F32)
        nc.sync.dma_start(out=winit[0:C_IN, :], in_=winit_in[:, :])
        wout = singles.tile([128, C_OUT], F32)
        nc.sync.dma_start(out=wout[0:64, :], in_=wout_in[:, :])
        bvec = singles.tile([128, 4], F32)
        for r in range(4):
            nc.sync.dma_start(out=bvec[:, r:r + 1],
                              in_=bvec_in[r:r + 1, :].rearrange("o p -> p o"))
        fi_rep = singles.tile([128, BC], F32)
        nc.sync.dma_start(out=fi_rep[0:64, :], in_=fi_in[:, :])

        zT = singles.tile([128, BC], F32)
        nc.vector.memset(zT[0:64, :], 0.0)

        def load_chunk(chk):
            cft = cf_pool.tile([128, CHUNK * 4 * BC], F32, name="cft", tag="cft")
            nc.sync.dma_start(
                out=cft[:, :].rearrange("c (t e b) -> c t e b", t=CHUNK, e=4),
                in_=cf_in[:, chk * CHUNK:(chk + 1) * CHUNK, :, :],
            )
            return cft

        cft = load_chunk(0)

        # ---- z0 ----
        p0 = p1pool.tile([128, BC], F32, name="p1t", tag="p1")
        nc.tensor.matmul(p0[0:64, :], winit[0:C_IN, :], a0t[0:C_IN, :],
                         start=True, stop=True, tile_position=(0, 0))
        z = zpool.tile([128, BC], F32, name="z", tag="z")
        nc.scalar.activation(z[0:64, :], p0[0:64, :], AFT.Identity,
                             bias=bvec[0:64, 2:3])
        g0 = gpool.tile([128, BC], F32, name="g", tag="g")
        nc.vector._custom_dve(MASKSEL, out=g0[0:64, :], in0=z[0:64, :],
                              in1=fi_rep[0:64, :], s0=0.0)
        nc.vector.tensor_add(zT[0:64, :], zT[0:64, :], g0[0:64, :])

        k4_prev = None

        b1bias = bvec[0:128, 0:1]

        # "front half" of a step: open its p1 bank with the early zp2 spine
        # matmul and RMW-add the E0 term on DVE. Pre-emitted during eval 4 of
        # the PREVIOUS step (right after zp2 is computed, before any DVE op
        # that waits on tanh_k4) so both stay off the critical path and the
        # only path op at the step boundary is the W1z/8 @ k4 matmul.
        _loaded_first = [False]

        def front_half(step_idx, zsrc):
            nonlocal cft
            if step_idx % CHUNK == 0 and _loaded_first[0]:
                cft = load_chunk(step_idx // CHUNK)
            _loaded_first[0] = True
            base = (step_idx % CHUNK) * 4 * BC
            p1n = p1pool.tile([128, BC], F32, name="p1t", tag="p1")
            nc.tensor.matmul(p1n[:, :], W_1[0:64, :], zsrc[0:64, :],
                             start=True, stop=False, tile_position=(0, 0))
            nc.vector.tensor_add(p1n[:, :], p1n[:, :],
                                 cft[:, base:base + BC])
            return p1n, cft, base

        p1_state = front_half(0, z)   # step 0: zp2_prev := z0, no k4 term

        for _rep in range(repeats):
            for t in range(T):
                p1, bcft, base = p1_state

                def xs(e, _c=bcft, _b=base):
                    return _c[:, _b + e * BC: _b + (e + 1) * BC]

                def add_E(e):
                    nc.vector.tensor_add(p1[:, :], p1[:, :], xs(e))

                if k4_prev is not None:
                    nc.tensor.matmul(p1[:, :], W_18[0:64, :], k4_prev[0:64, :],
                                     start=False, stop=False, tile_position=(0, 0))

                k = [None] * 4
                h = [None] * 4
                q = [None] * 4

                # ---- eval 1 ----
                h[0] = hpool.tile([128, BC], F32, name="h", tag="h")
                nc.scalar.activation(h[0][:, :], p1[:, :], AFT.Tanh, bias=b1bias)
                q[0] = p2pool.tile([128, BC], F32, name="p2t", tag="p2")
                nc.tensor.matmul(q[0][0:64, :], w2[:, :], h[0][:, :],
                                 start=True, stop=True, tile_position=(0, 0))
                add_E(1)
                k[0] = kpool.tile([128, BC], F32, name="k1", tag="k1")
                nc.scalar.activation(k[0][0:64, :], q[0][0:64, :], AFT.Tanh,
                                     bias=bvec[0:64, 1:2])
                zu = hpool.tile([128, BC], F32, name="zu", tag="zu")
                nc.vector._custom_dve(AXPY, out=zu[0:64, :], in0=z[0:64, :],
                                      in1=k[0][0:64, :], s0=0.125)
                # ---- eval 2 ----
                nc.tensor.matmul(p1[:, :], W_13[0:64, :], k[0][0:64, :],
                                 start=False, stop=False, tile_position=(0, 0))
                h[1] = hpool.tile([128, BC], F32, name="h", tag="h")
                nc.scalar.activation(h[1][:, :], p1[:, :], AFT.Tanh, bias=b1bias)
                q[1] = p2pool.tile([128, BC], F32, name="p2t", tag="p2")
                nc.tensor.matmul(q[1][0:64, :], w2[:, :], h[1][:, :],
                                 start=True, stop=True, tile_position=(0, 0))
                add_E(2)
                nc.tensor.matmul(p1[:, :], W_M23[0:64, :], k[0][0:64, :],
                                 start=False, stop=False, tile_position=(0, 0))
                k[1] = kpool.tile([128, BC], F32, name="k2", tag="k2")
                nc.scalar.activation(k[1][0:64, :], q[1][0:64, :], AFT.Tanh,
                                     bias=bvec[0:64, 1:2])
                zv = hpool.tile([128, BC], F32, name="zv", tag="zv")
                nc.vector._custom_dve(AXPY, out=zv[0:64, :], in0=zu[0:64, :],
                                      in1=k[1][0:64, :], s0=0.375)
                # ---- eval 3 ----
                nc.tensor.matmul(p1[:, :], W_1[0:64, :], k[1][0:64, :],
                                 start=False, stop=False, tile_position=(0, 0))
                h[2] = hpool.tile([128, BC], F32, name="h", tag="h")
                nc.scalar.activation(h[2][:, :], p1[:, :], AFT.Tanh, bias=b1bias)
                q[2] = p2pool.tile([128, BC], F32, name="p2t", tag="p2")
                nc.tensor.matmul(q[2][0:64, :], w2[:, :], h[2][:, :],
                                 start=True, stop=True, tile_position=(0, 0))
                add_E(3)
                nc.tensor.matmul(p1[:, :], W_43[0:64, :], k[0][0:64, :],
                                 start=False, stop=False, tile_position=(0, 0))
                nc.tensor.matmul(p1[:, :], W_M2[0:64, :], k[1][0:64, :],
                                 start=False, stop=False, tile_position=(0, 0))
                k[2] = kpool.tile([128, BC], F32, name="k3", tag="k3")
                nc.scalar.activation(k[2][0:64, :], q[2][0:64, :], AFT.Tanh,
                                     bias=bvec[0:64, 1:2])
                # zp2 = z + (k1+3k2+3k3)/8, built incrementally (zu, zv above)
                zp2 = zpool.tile([128, BC], F32, name="zp2", tag="zp2")
                nc.vector._custom_dve(AXPY, out=zp2[0:64, :], in0=zv[0:64, :],
                                      in1=k[2][0:64, :], s0=0.375)
                # ---- eval 4 ----
                nc.tensor.matmul(p1[:, :], W_1[0:64, :], k[2][0:64, :],
                                 start=False, stop=True, tile_position=(0, 0))
                h[3] = hpool.tile([128, BC], F32, name="h", tag="h")
                nc.scalar.activation(h[3][:, :], p1[:, :], AFT.Tanh, bias=b1bias)
                q[3] = p2pool.tile([128, BC], F32, name="p2t", tag="p2")
                nc.tensor.matmul(q[3][0:64, :], w2[:, :], h[3][:, :],
                                 start=True, stop=True, tile_position=(0, 0))
                # front half issued after the path W2 matmul: keeps the PE
                # in-order queue from stalling W2.h4 behind W_1.zp2 if the
                # DVE zp2 chain runs late.
                if not (_rep == repeats - 1 and t == T - 1):
                    p1_state = front_half(t + 1 if t + 1 < T else 0, zp2)
                k[3] = kpool.tile([128, BC], F32, name="k4", tag="k4")
                nc.scalar.activation(k[3][0:64, :], q[3][0:64, :], AFT.Tanh,
                                     bias=bvec[0:64, 1:2])
                # z' = zp2 + k4/8 ; gather
                znew = zpool.tile([128, BC], F32, name="z", tag="z")
                nc.vector._custom_dve(AXPY, out=znew[0:64, :], in0=zp2[0:64, :],
                                      in1=k[3][0:64, :], s0=0.125)
                g = gpool.tile([128, BC], F32, name="g", tag="g")
                nc.vector._custom_dve(MASKSEL, out=g[0:64, :], in0=znew[0:64, :],
                                      in1=fi_rep[0:64, :], s0=float(t + 1))
                nc.vector.tensor_add(zT[0:64, :], zT[0:64, :], g[0:64, :])

                z = znew
                k4_prev = k[3]

        # ---- readout ----
        po = p2pool.tile([128, BC], F32, name="po", tag="p2")
        nc.tensor.matmul(po[0:C_OUT, :], wout[0:64, :], zT[0:64, :],
                         start=True, stop=True, tile_position=(0, 0))
        ot = singles.tile([128, BC], F32)
        nc.scalar.activation(ot[0:C_OUT, :], po[0:C_OUT, :], AFT.Identity,
                             bias=bvec[0:C_OUT, 3:4])
        nc.sync.dma_start(out=out_ext[:, :], in_=ot[0:C_OUT, :])

    if prune:
        _prune_sync(nc)
    nc.compile()
    return nc


def prepare_inputs(times, coeff_a, coeff_b, coeff_two_c, coeff_three_d,
                   final_index, W_init, b_init, W1, b1, W2, b2, W_out, b_out,
                   T=T_FULL):
    fi = np.asarray(final_index).astype(np.int64)
    W1 = np.asarray(W1, np.float64)
    b1 = np.asarray(b1, np.float32)
    W2_ = np.asarray(W2, np.float32)
    b2_ = np.asarray(b2, np.float32)
    W_init_ = np.asarray(W_init, np.float32)
    b_init_ = np.asarray(b_init, np.float32)
    W_out_ = np.asarray(W_out, np.float32)
    b_out_ = np.asarray(b_out, np.float32)

    A, X13, X23 = _spline_tables(times, coeff_a, coeff_b, coeff_two_c,
                                 coeff_three_d)
    b1_nonzero = bool(np.any(b1 != 0))
    n_chunks = (T + CHUNK - 1) // CHUNK
    t_pad = n_chunks * CHUNK

    At = np.transpose(A, (2, 1, 0))
    X13t = np.transpose(X13, (2, 1, 0))
    X23t = np.transpose(X23, (2, 1, 0))
    Xd = np.zeros((C_IN, t_pad, 4, B), np.float64)
    Xd[:, :T, 0] = At[:, :T]
    Xd[:, :T, 1] = (X13t - At[:, :L - 1])[:, :T]
    Xd[:, :T, 2] = (X23t - X13t)[:, :T]
    Xd[:, :T, 3] = (At[:, 1:] - X23t)[:, :T]
    W1x64 = W1[C_HID:]
    E = np.einsum("cteb,ch->hteb", Xd, W1x64, optimize=True)
    cf_all = np.ascontiguousarray(E, np.float32)
    a0_all = np.ascontiguousarray(At[:, 0], np.float32)

    W1z = W1[:C_HID]
    W1x_ = np.ascontiguousarray(W1[C_HID:].astype(np.float32))

    wz = np.stack([W1z, W1z / 8.0, W1z / 3.0, -2.0 / 3.0 * W1z,
                   4.0 / 3.0 * W1z, -2.0 * W1z]).astype(np.float32)

    bvec = np.zeros((4, 128), np.float32)
    bvec[0, :C_HH] = b1
    bvec[1, :C_HID] = b2_
    bvec[2, :C_HID] = b_init_
    bvec[3, :C_OUT] = b_out_

    in_maps = []
    for core in range(N_CORES):
        cols = slice(core * BC, (core + 1) * BC)
        in_maps.append({
            "cf": np.ascontiguousarray(cf_all[:, :, :, cols]),
            "a0": np.ascontiguousarray(a0_all[:, cols]),
            "wz": wz,
            "w1x": W1x_,
            "w2": W2_,
            "winit": W_init_,
            "wout": W_out_,
            "bvec": bvec,
            "fi": np.ascontiguousarray(
                np.broadcast_to(fi[cols].astype(np.float32), (C_HID, BC))),
        })
    return in_maps, b1_nonzero


_PROGRAM_CACHE = {}


def run(inputs, T=T_FULL, trace=False):
    in_maps, b1_nonzero = prepare_inputs(T=T, **inputs)
    key = (T, b1_nonzero)
    if key not in _PROGRAM_CACHE:
        _PROGRAM_CACHE[key] = build_program(T=T, b1_nonzero=b1_nonzero)
    nc = _PROGRAM_CACHE[key]
    res = run_bass_kernel_spmd(nc, in_maps, core_ids=list(range(N_CORES)),
                               trace=trace)
    outs = [res.results[c]["out"] for c in range(N_CORES)]
    full = np.concatenate([o.T for o in outs], axis=0).astype(np.float32)
    return full, res


def kernel(**inputs):
    out, _ = run(inputs)
    return out

